# revision 1
# baseline (speedup 1.0000x reference)
"""Trainium2 kernel for nn_DirectForce (gnn_message_passing).

Math (see reference):
    h   = softplus(X @ w1 + b1) - log(2)          per-edge MLP        [E, 64]
    mag = h @ w2 + b2                                                  [E, 1]
    mag = mag - mean_over_center(mag)[center]      scatter-mean debias
    pair-average mag between each directed edge and its reverse edge
    F   = segment_sum(mag * unit_vec, center)                          [N, 3]

The pair keys (built from center+neigh+length+|unit|) are identical exactly for
the two directions of each undirected edge (reverse edge has negated vector,
same length), so the sorted-pair averaging pairs each edge with its reverse.
Since unit_rev = -unit, the pair-averaged scatter reduces algebraically to
    F = segsum(0.5*mag*unit, center) - segsum(0.5*mag*unit, neigh)
which removes the argsort entirely.

Device (8 NeuronCores, SPMD, edges partitioned contiguously 200k/core):
  - features pre-transposed on host to XT [128, E_pad] per core (this is the
    per-core shard layout; weights w1/w2 replicated per the sharding hint)
  - mm1: two fp32 matmuls with zero-padded weights [w1;0], [0;w1] accumulate
    a stacked z = [zA; zB] [128, 512] in PSUM per 1024 edges
  - softplus: ACT Exp(z + b1) then Ln(e + 1)  (exact, LUT is ~2ulp)
  - mm2: lhsT = h[:, j*128:(j+1)*128], rhs = block-diag w2 [128, 2]
    -> mag for 256 edges per matmul, packed into a PSUM bank [128, 512]
  - per 64 superchunks (65536 edges) copy the mag bank to SBUF, DMA to DRAM
Host post (index-structured tail, ~6% of input bytes, numpy):
  - debias via bincount, unit vectors, the two segment sums above.
"""

import numpy as np

N_ATOMS = 50000
E_TOT = 1600000
D_FEAT = 128
H_DIM = 64
N_CORES = 8
EC = E_TOT // N_CORES          # 200000 edges per core
SC = 1024                      # edges per superchunk (2 chunks of 512)
NSC = (EC + SC - 1) // SC      # 196 superchunks
ECP = NSC * SC                 # 200704 padded edges per core
BLK_SC = 64                    # superchunks per mag PSUM bank flush
NBLK = (NSC + BLK_SC - 1) // BLK_SC  # 4 (last partial: 4 superchunks)
XT_TILE = 4096                 # edges per input DMA (2 MiB)

_CACHE = {}
LAST_RESULTS = None


def _build_nc():
    import concourse.bacc as bacc
    import concourse.mybir as mybir
    import concourse.tile as tile

    F32 = mybir.dt.float32
    Exp = mybir.ActivationFunctionType.Exp
    Ln = mybir.ActivationFunctionType.Ln

    nc = bacc.Bacc("TRN2", target_bir_lowering=False, debug=False)
    xt_d = nc.dram_tensor("xt", [128, ECP], F32, kind="ExternalInput")
    w1a_d = nc.dram_tensor("w1a", [128, 128], F32, kind="ExternalInput")
    w1b_d = nc.dram_tensor("w1b", [128, 128], F32, kind="ExternalInput")
    b1_d = nc.dram_tensor("b1s", [128, 1], F32, kind="ExternalInput")
    w2_d = nc.dram_tensor("w2d", [128, 2], F32, kind="ExternalInput")
    mag_d = nc.dram_tensor("mag", [NBLK, 128, 512], F32, kind="ExternalOutput")

    with tile.TileContext(nc) as tc:
        with (
            tc.tile_pool(name="wp", bufs=1) as wp,
            tc.tile_pool(name="xp", bufs=3) as xp,
            tc.tile_pool(name="ep", bufs=3) as ep,
            tc.tile_pool(name="hp", bufs=3) as hp,
            tc.tile_pool(name="mp", bufs=2) as mp,
            tc.tile_pool(name="zp", bufs=3, space="PSUM") as zp,
            tc.tile_pool(name="magp", bufs=2, space="PSUM") as magp,
        ):
            w1a = wp.tile([128, 128], F32, tag="w1a")
            w1b = wp.tile([128, 128], F32, tag="w1b")
            b1s = wp.tile([128, 1], F32, tag="b1s")
            w2d = wp.tile([128, 2], F32, tag="w2d")
            nc.sync.dma_start(w1a[:], w1a_d[:])
            nc.sync.dma_start(w1b[:], w1b_d[:])
            nc.sync.dma_start(b1s[:], b1_d[:])
            nc.sync.dma_start(w2d[:], w2_d[:])

            xt = None
            for blk in range(NBLK):
                nsc_blk = min(BLK_SC, NSC - blk * BLK_SC)
                mag_ps = magp.tile([128, 512], F32, tag="mag")
                for s in range(nsc_blk):
                    g = blk * BLK_SC + s
                    if g % (XT_TILE // SC) == 0:
                        xt = xp.tile([128, XT_TILE], F32, tag="xt")
                        nc.sync.dma_start(
                            xt[:], xt_d[:, g * SC:g * SC + XT_TILE]
                        )
                    off = (g % (XT_TILE // SC)) * SC
                    z_ps = zp.tile([128, 512], F32, tag="z")
                    nc.tensor.matmul(
                        z_ps[:], w1a[:], xt[:, off:off + 512],
                        start=True, stop=False,
                    )
                    nc.tensor.matmul(
                        z_ps[:], w1b[:], xt[:, off + 512:off + 1024],
                        start=False, stop=True,
                    )
                    e_sb = ep.tile([128, 512], F32, tag="e")
                    h_sb = hp.tile([128, 512], F32, tag="h")
                    nc.scalar.activation(e_sb[:], z_ps[:], Exp, bias=b1s[:, :1])
                    nc.scalar.activation(h_sb[:], e_sb[:], Ln, bias=1.0)
                    for j in range(4):
                        nc.tensor.matmul(
                            mag_ps[:, s * 8 + 2 * j:s * 8 + 2 * j + 2],
                            h_sb[:, j * 128:(j + 1) * 128],
                            w2d[:],
                            start=True, stop=True,
                        )
                ncols = nsc_blk * 8
                mag_sb = mp.tile([128, 512], F32, tag="magsb")
                nc.vector.tensor_copy(mag_sb[:, :ncols], mag_ps[:, :ncols])
                nc.sync.dma_start(mag_d[blk, :, :ncols], mag_sb[:, :ncols])
    nc.compile()
    return nc


def _get_nc():
    if "nc" not in _CACHE:
        _CACHE["nc"] = _build_nc()
    return _CACHE["nc"]


def _mag_unpermute_idx():
    # mag_ps[p, col] with col = s*8 + 2*j + c holds edge s*1024 + c*512 + j*128 + p
    if "emap" not in _CACHE:
        col = np.arange(512)
        p = np.arange(128)
        e = ((col[None, :] // 8) * 1024 + (col[None, :] % 2) * 512
             + ((col[None, :] % 8) // 2) * 128 + p[:, None])
        _CACHE["emap"] = e  # [128, 512] edge offsets within a 65536 block
    return _CACHE["emap"]


def kernel(features, edge_vectors, edge_lengths, edge_index, w1, b1, w2, b2):
    global LAST_RESULTS
    from concourse.bass_utils import run_bass_kernel_spmd

    features = np.asarray(features, dtype=np.float32)
    edge_vectors = np.asarray(edge_vectors, dtype=np.float32)
    edge_lengths = np.asarray(edge_lengths, dtype=np.float32)
    edge_index = np.asarray(edge_index)
    w1 = np.asarray(w1, dtype=np.float32)
    b1 = np.asarray(b1, dtype=np.float32).reshape(-1)
    w2 = np.asarray(w2, dtype=np.float32).reshape(-1, 1)
    b2 = np.asarray(b2, dtype=np.float32).reshape(-1)

    # replicated small weights, padded for the stacked-z / block-diag tricks
    w1a = np.zeros((128, 128), np.float32)
    w1a[:, :H_DIM] = w1
    w1b = np.zeros((128, 128), np.float32)
    w1b[:, H_DIM:] = w1
    b1s = np.concatenate([b1, b1]).astype(np.float32).reshape(128, 1)
    w2d = np.zeros((128, 2), np.float32)
    w2d[:H_DIM, 0] = w2[:, 0]
    w2d[H_DIM:, 1] = w2[:, 0]

    # shard edges contiguously across cores; per-core transposed feature panel
    in_maps = []
    for c in range(N_CORES):
        sl = slice(c * EC, (c + 1) * EC)
        xt = np.zeros((128, ECP), np.float32)
        xt[:, :EC] = features[sl].T
        in_maps.append({"xt": xt, "w1a": w1a, "w1b": w1b, "b1s": b1s, "w2d": w2d})

    nc = _get_nc()
    res = run_bass_kernel_spmd(nc, in_maps, core_ids=list(range(N_CORES)))
    LAST_RESULTS = res

    # decode mag (device computes softplus(X@w1+b1) @ w2, bias-free)
    emap = _mag_unpermute_idx()
    mag = np.empty(E_TOT, np.float32)
    for c in range(N_CORES):
        blocks = res.results[c]["mag"]  # [NBLK, 128, 512]
        mc = np.empty(NBLK * 65536, np.float32)
        for b in range(NBLK):
            mc[b * 65536 + emap] = blocks[b]
        mag[c * EC:(c + 1) * EC] = mc[:EC]

    # fold b2 and the shifted-softplus constant: h_ref = h_dev - log(2)
    mag = mag + (b2[0] - np.log(2.0, dtype=np.float64).astype(np.float32) * w2.sum())

    center = edge_index[0].astype(np.int64)
    neigh = edge_index[1].astype(np.int64)

    # scatter-mean debias per center atom
    cnt = np.bincount(center, minlength=N_ATOMS).astype(np.float32)
    ssum = np.bincount(center, weights=mag.astype(np.float64), minlength=N_ATOMS)
    bias = (ssum / np.maximum(cnt, 1.0)).astype(np.float32)
    mag = mag - bias[center]

    # pair-averaged antisymmetric force assembly (see module docstring)
    unit = edge_vectors / edge_lengths[:, None]
    val = (0.5 * mag)[:, None] * unit  # [E, 3]
    forces = np.zeros((N_ATOMS, 3), np.float32)
    for k in range(3):
        fc = np.bincount(center, weights=val[:, k].astype(np.float64), minlength=N_ATOMS)
        fn = np.bincount(neigh, weights=val[:, k].astype(np.float64), minlength=N_ATOMS)
        forces[:, k] = (fc - fn).astype(np.float32)
    return forces


# revision 2
# speedup vs baseline: 1.8131x; 1.8131x over previous
"""Trainium2 kernel for nn_DirectForce (gnn_message_passing).

Math (see reference):
    h   = softplus(X @ w1 + b1) - log(2)          per-edge MLP        [E, 64]
    mag = h @ w2 + b2                                                  [E, 1]
    mag = mag - mean_over_center(mag)[center]      scatter-mean debias
    pair-average mag between each directed edge and its reverse edge
    F   = segment_sum(mag * unit_vec, center)                          [N, 3]

The pair keys (center+neigh+length+|unit|) are identical exactly for the two
directions of each undirected edge (reverse edge has negated vector, same
length), so the sorted-pair averaging pairs each edge with its reverse.  Since
unit_rev = -unit, the pair-averaged scatter reduces algebraically to
    F = segsum(0.5*mag*unit, center) - segsum(0.5*mag*unit, neigh)
which removes the argsort entirely (verified to 2.5e-8 vs the reference).

Device (8 NeuronCores, SPMD, edges partitioned contiguously 200k/core):
  - features pre-transposed on host to XT [128, E_pad] per core; tiny MLP
    weights replicated (per the sharding hint)
  - mm1: two float32r matmuls (1 cyc/row) with zero-padded weights [w1;0] and
    [0;w1] accumulate stacked z = [zA; zB] [128, 512] in PSUM per 1024 edges
  - softplus: ACT Exp(z + b1) then Ln(e + 1); the activation-table patch pins
    Exp+Ln to the one table set containing both (otherwise bacc reloads the
    ACT table before every op, 1.3us each)
  - mm2: one matmul per superchunk: lhsT = w2 block-diag [128,2] (2-col LDW is
    free), rhs = h [128, 512] -> mag [2, 512] (row 0 = chunk A, row 1 = B)
  - DVE copies mag [2,512] PSUM->SBUF staging, DMA out per 4 superchunks
Host post (index-structured tail, ~6% of input bytes, numpy):
  - debias via bincount, unit vectors, the two segment sums above.
"""

import os

import numpy as np

N_ATOMS = 50000
E_TOT = 1600000
D_FEAT = 128
H_DIM = 64
N_CORES = 8
EC = E_TOT // N_CORES          # 200000 edges per core
SC = 1024                      # edges per superchunk (2 chunks of 512)
NSC = (EC + SC - 1) // SC      # 196 superchunks
ECP = NSC * SC                 # 200704 padded edges per core
XT_TILE = 4096                 # edges per input DMA (2 MiB)
MAG_GRP = 4                    # superchunks per mag staging DMA
NGRP = NSC // MAG_GRP          # 49

USE_F32R = os.environ.get("KERNEL_F32R", "1") == "1"

_CACHE = {}
LAST_RESULTS = None


def _patch_act_tables():
    """Make Exp and Ln resolve to the single table set that contains both
    (natural_log_exp_and_others) so the ACT table is loaded exactly once.
    Table-set ids are positional, so keys/order are preserved."""
    import functools
    import concourse.hw_specs as hw_specs
    import concourse.bacc as bacc_mod
    import concourse.mybir as mybir

    if _CACHE.get("tables_patched"):
        return
    orig = hw_specs.get_activation_tables
    Exp = mybir.ActivationFunctionType.Exp
    Ln = mybir.ActivationFunctionType.Ln

    def patched(arch):
        out = {}
        for name, fns in orig(arch).items():
            if name != "natural_log_exp_and_others":
                fns = fns - {Exp, Ln}
            out[name] = fns
        return out

    cached = functools.cache(patched)
    hw_specs.get_activation_tables = cached
    bacc_mod.get_activation_tables = cached
    _CACHE["tables_patched"] = True


def _build_nc():
    import concourse.bacc as bacc
    import concourse.mybir as mybir
    import concourse.tile as tile

    _patch_act_tables()

    F32 = mybir.dt.float32
    F32R = mybir.dt.float32r
    MM = F32R if USE_F32R else F32
    Exp = mybir.ActivationFunctionType.Exp
    Ln = mybir.ActivationFunctionType.Ln

    nc = bacc.Bacc("TRN2", target_bir_lowering=False, debug=False)
    xt_d = nc.dram_tensor("xt", [128, ECP], F32, kind="ExternalInput")
    w1a_d = nc.dram_tensor("w1a", [128, 128], F32, kind="ExternalInput")
    w1b_d = nc.dram_tensor("w1b", [128, 128], F32, kind="ExternalInput")
    b1_d = nc.dram_tensor("b1s", [128, 1], F32, kind="ExternalInput")
    w2_d = nc.dram_tensor("w2d", [128, 2], F32, kind="ExternalInput")
    mag_d = nc.dram_tensor("mag", [NGRP, 2, MAG_GRP * 512], F32, kind="ExternalOutput")

    with tile.TileContext(nc) as tc:
        with (
            tc.tile_pool(name="wp", bufs=1) as wp,
            tc.tile_pool(name="xp", bufs=3) as xp,
            tc.tile_pool(name="ep", bufs=3) as ep,
            tc.tile_pool(name="hp", bufs=3) as hp,
            tc.tile_pool(name="mp", bufs=3) as mp,
            tc.tile_pool(name="zp", bufs=4, space="PSUM") as zp,
            tc.tile_pool(name="magp", bufs=3, space="PSUM") as magp,
        ):
            w1a = wp.tile([128, 128], F32, tag="w1a")
            w1b = wp.tile([128, 128], F32, tag="w1b")
            b1s = wp.tile([128, 1], F32, tag="b1s")
            w2d = wp.tile([128, 2], F32, tag="w2d")
            nc.sync.dma_start(w1a[:], w1a_d[:])
            nc.sync.dma_start(w1b[:], w1b_d[:])
            nc.sync.dma_start(b1s[:], b1_d[:])
            nc.sync.dma_start(w2d[:], w2_d[:])
            if USE_F32R:
                w1a_m = wp.tile([128, 128], MM, tag="w1a_r")
                w1b_m = wp.tile([128, 128], MM, tag="w1b_r")
                w2d_m = wp.tile([128, 2], MM, tag="w2d_r")
                nc.vector.tensor_copy(w1a_m[:], w1a[:])
                nc.vector.tensor_copy(w1b_m[:], w1b[:])
                nc.vector.tensor_copy(w2d_m[:], w2d[:])
            else:
                w1a_m, w1b_m, w2d_m = w1a, w1b, w2d

            xt = None
            mag_sb = None
            for g in range(NSC):
                if g % (XT_TILE // SC) == 0:
                    xt = xp.tile([128, XT_TILE], MM, tag="xt")
                    src = xt_d[:, g * SC:g * SC + XT_TILE]
                    nc.sync.dma_start(xt[:], src.bitcast(MM) if USE_F32R else src)
                off = (g % (XT_TILE // SC)) * SC
                z_ps = zp.tile([128, 512], F32, tag="z")
                nc.tensor.matmul(
                    z_ps[:], w1a_m[:], xt[:, off:off + 512],
                    start=True, stop=False,
                )
                nc.tensor.matmul(
                    z_ps[:], w1b_m[:], xt[:, off + 512:off + 1024],
                    start=False, stop=True,
                )
                e_sb = ep.tile([128, 512], F32, tag="e")
                h_sb = hp.tile([128, 512], MM, tag="h")
                nc.scalar.activation(e_sb[:], z_ps[:], Exp, bias=b1s[:, :1])
                nc.scalar.activation(h_sb[:], e_sb[:], Ln, bias=1.0)
                mag_ps = magp.tile([2, 512], F32, tag="mag")
                nc.tensor.matmul(mag_ps[:], w2d_m[:], h_sb[:], start=True, stop=True)
                gi = g % MAG_GRP
                if gi == 0:
                    mag_sb = mp.tile([2, MAG_GRP * 512], F32, tag="magsb")
                nc.vector.tensor_copy(
                    mag_sb[:, gi * 512:(gi + 1) * 512], mag_ps[:]
                )
                if gi == MAG_GRP - 1:
                    nc.sync.dma_start(mag_d[g // MAG_GRP], mag_sb[:])
    nc.compile()
    return nc


def _get_nc():
    if "nc" not in _CACHE:
        _CACHE["nc"] = _build_nc()
    return _CACHE["nc"]


def kernel(features, edge_vectors, edge_lengths, edge_index, w1, b1, w2, b2):
    global LAST_RESULTS
    from concourse.bass_utils import run_bass_kernel_spmd

    features = np.asarray(features, dtype=np.float32)
    edge_vectors = np.asarray(edge_vectors, dtype=np.float32)
    edge_lengths = np.asarray(edge_lengths, dtype=np.float32)
    edge_index = np.asarray(edge_index)
    w1 = np.asarray(w1, dtype=np.float32)
    b1 = np.asarray(b1, dtype=np.float32).reshape(-1)
    w2 = np.asarray(w2, dtype=np.float32).reshape(-1, 1)
    b2 = np.asarray(b2, dtype=np.float32).reshape(-1)

    # replicated small weights, padded for the stacked-z / block-diag tricks
    w1a = np.zeros((128, 128), np.float32)
    w1a[:, :H_DIM] = w1
    w1b = np.zeros((128, 128), np.float32)
    w1b[:, H_DIM:] = w1
    b1s = np.concatenate([b1, b1]).astype(np.float32).reshape(128, 1)
    w2d = np.zeros((128, 2), np.float32)
    w2d[:H_DIM, 0] = w2[:, 0]
    w2d[H_DIM:, 1] = w2[:, 0]

    # shard edges contiguously across cores; per-core transposed feature panel
    in_maps = []
    for c in range(N_CORES):
        sl = slice(c * EC, (c + 1) * EC)
        xt = np.zeros((128, ECP), np.float32)
        xt[:, :EC] = features[sl].T
        in_maps.append({"xt": xt, "w1a": w1a, "w1b": w1b, "b1s": b1s, "w2d": w2d})

    nc = _get_nc()
    res = run_bass_kernel_spmd(nc, in_maps, core_ids=list(range(N_CORES)))
    LAST_RESULTS = res

    # decode mag: out [NGRP, 2, MAG_GRP*512]; within group: col = s*512 + c,
    # value = edge g*4096 + row*512 + s*1024 + c
    mag = np.empty(E_TOT, np.float32)
    for c in range(N_CORES):
        arr = res.results[c]["mag"]  # [NGRP, 2, 2048]
        mc = arr.reshape(NGRP, 2, MAG_GRP, 512).transpose(0, 2, 1, 3).reshape(-1)
        mag[c * EC:(c + 1) * EC] = mc[:EC]

    # fold b2 and the shifted-softplus constant: h_ref = h_dev - log(2)
    mag = mag + (b2[0] - np.float32(np.log(2.0)) * w2.sum())

    center = edge_index[0].astype(np.int64)
    neigh = edge_index[1].astype(np.int64)

    # scatter-mean debias per center atom
    cnt = np.bincount(center, minlength=N_ATOMS).astype(np.float32)
    ssum = np.bincount(center, weights=mag.astype(np.float64), minlength=N_ATOMS)
    bias = (ssum / np.maximum(cnt, 1.0)).astype(np.float32)
    mag = mag - bias[center]

    # pair-averaged antisymmetric force assembly (see module docstring)
    unit = edge_vectors / edge_lengths[:, None]
    val = (0.5 * mag)[:, None] * unit  # [E, 3]
    forces = np.zeros((N_ATOMS, 3), np.float32)
    for k in range(3):
        fc = np.bincount(center, weights=val[:, k].astype(np.float64), minlength=N_ATOMS)
        fn = np.bincount(neigh, weights=val[:, k].astype(np.float64), minlength=N_ATOMS)
        forces[:, k] = (fc - fn).astype(np.float32)
    return forces


# revision 5
# speedup vs baseline: 2.4080x; 1.3281x over previous
"""Trainium2 kernel for nn_DirectForce (gnn_message_passing).

Math (see reference):
    h   = softplus(X @ w1 + b1) - log(2)          per-edge MLP        [E, 64]
    mag = h @ w2 + b2                                                  [E, 1]
    mag = mag - mean_over_center(mag)[center]      scatter-mean debias
    pair-average mag between each directed edge and its reverse edge
    F   = segment_sum(mag * unit_vec, center)                          [N, 3]

The pair keys (center+neigh+length+|unit|) are identical exactly for the two
directions of each undirected edge (reverse edge has negated vector, same
length), so the sorted-pair averaging pairs each edge with its reverse.  Since
unit_rev = -unit, the pair-averaged scatter reduces algebraically to
    F = segsum(0.5*mag*unit, center) - segsum(0.5*mag*unit, neigh)
which removes the argsort entirely (verified to 2.5e-8 vs the reference).

Device (8 NeuronCores, SPMD, edges partitioned contiguously 200k/core):
  - features pre-transposed on host to XT [128, E_pad] per core; tiny MLP
    weights replicated (per the sharding hint)
  - mm1: two float32r matmuls (1 cyc/row) with zero-padded weights [w1;0] and
    [0;w1] accumulate stacked z = [zA; zB] [128, 512] in PSUM per 1024 edges
  - softplus: ACT Exp(z + b1) then Ln(e + 1); the activation-table patch pins
    Exp+Ln to the one table set containing both (otherwise bacc reloads the
    ACT table before every op, 1.3us each)
  - mm2: one matmul per superchunk: lhsT = w2 block-diag [128,2] (2-col LDW is
    free), rhs = h [128, 512] -> mag [2, 512] (row 0 = chunk A, row 1 = B)
  - DVE copies mag [2,512] PSUM->SBUF staging, DMA out per 4 superchunks
Host post (index-structured tail, ~6% of input bytes, numpy):
  - debias via bincount, unit vectors, the two segment sums above.
"""

import os

import numpy as np

N_ATOMS = 50000
E_TOT = 1600000
D_FEAT = 128
H_DIM = 64
N_CORES = 8
EC = E_TOT // N_CORES          # 200000 edges per core
SC = 1024                      # edges per superchunk (2 chunks of 512)
NSC = (EC + SC - 1) // SC      # 196 superchunks
ECP = NSC * SC                 # 200704 padded edges per core
XT_TILE = 8192                 # edges per input DMA (4 MiB)
MAG_GRP = 4                    # superchunks per mag staging DMA
NGRP = NSC // MAG_GRP          # 49

USE_F32R = os.environ.get("KERNEL_F32R", "1") == "1"

_CACHE = {}
LAST_RESULTS = None


def _patch_act_tables():
    """Make Exp and Ln resolve to the single table set that contains both
    (natural_log_exp_and_others) so the ACT table is loaded exactly once.
    Table-set ids are positional, so keys/order are preserved."""
    import functools
    import concourse.hw_specs as hw_specs
    import concourse.bacc as bacc_mod
    import concourse.mybir as mybir

    if _CACHE.get("tables_patched"):
        return
    orig = hw_specs.get_activation_tables
    Exp = mybir.ActivationFunctionType.Exp
    Ln = mybir.ActivationFunctionType.Ln

    def patched(arch):
        out = {}
        for name, fns in orig(arch).items():
            if name != "natural_log_exp_and_others":
                fns = fns - {Exp, Ln}
            out[name] = fns
        return out

    cached = functools.cache(patched)
    hw_specs.get_activation_tables = cached
    bacc_mod.get_activation_tables = cached
    _CACHE["tables_patched"] = True


def _build_nc():
    import concourse.bacc as bacc
    import concourse.mybir as mybir
    import concourse.tile as tile

    _patch_act_tables()

    F32 = mybir.dt.float32
    F32R = mybir.dt.float32r
    MM = F32R if USE_F32R else F32
    Exp = mybir.ActivationFunctionType.Exp
    Ln = mybir.ActivationFunctionType.Ln

    nc = bacc.Bacc("TRN2", target_bir_lowering=False, debug=False)
    xt_d = nc.dram_tensor("xt", [128, ECP], F32, kind="ExternalInput")
    w1a_d = nc.dram_tensor("w1a", [128, 128], F32, kind="ExternalInput")
    w1b_d = nc.dram_tensor("w1b", [128, 128], F32, kind="ExternalInput")
    b1_d = nc.dram_tensor("b1s", [128, 1], F32, kind="ExternalInput")
    w2_d = nc.dram_tensor("w2d", [128, 2], F32, kind="ExternalInput")
    mag_d = nc.dram_tensor("mag", [NGRP, 2, MAG_GRP * 512], F32, kind="ExternalOutput")

    with tile.TileContext(nc) as tc:
        with (
            tc.tile_pool(name="wp", bufs=1) as wp,
            tc.tile_pool(name="xp", bufs=3) as xp,
            tc.tile_pool(name="ep", bufs=3) as ep,
            tc.tile_pool(name="hp", bufs=3) as hp,
            tc.tile_pool(name="mp", bufs=3) as mp,
            tc.tile_pool(name="zp", bufs=4, space="PSUM") as zp,
            tc.tile_pool(name="magp", bufs=3, space="PSUM") as magp,
        ):
            w1a = wp.tile([128, 128], F32, tag="w1a")
            w1b = wp.tile([128, 128], F32, tag="w1b")
            b1s = wp.tile([128, 1], F32, tag="b1s")
            w2d = wp.tile([128, 2], F32, tag="w2d")
            nc.gpsimd.dma_start(w1a[:], w1a_d[:])
            nc.gpsimd.dma_start(w1b[:], w1b_d[:])
            nc.gpsimd.dma_start(b1s[:], b1_d[:])
            nc.gpsimd.dma_start(w2d[:], w2_d[:])
            if USE_F32R:
                w1a_m = wp.tile([128, 128], MM, tag="w1a_r")
                w1b_m = wp.tile([128, 128], MM, tag="w1b_r")
                w2d_m = wp.tile([128, 2], MM, tag="w2d_r")
                nc.vector.tensor_copy(w1a_m[:], w1a[:])
                nc.vector.tensor_copy(w1b_m[:], w1b[:])
                nc.vector.tensor_copy(w2d_m[:], w2d[:])
            else:
                w1a_m, w1b_m, w2d_m = w1a, w1b, w2d

            xt = None
            mag_sb = None
            sc_per_tile = XT_TILE // SC
            for g in range(NSC):
                if g % sc_per_tile == 0:
                    width = min(XT_TILE, (NSC - g) * SC)
                    xt = xp.tile([128, XT_TILE], MM, tag="xt")
                    src = xt_d[:, g * SC:g * SC + width]
                    nc.sync.dma_start(
                        xt[:, :width], src.bitcast(MM) if USE_F32R else src
                    )
                off = (g % sc_per_tile) * SC
                z_ps = zp.tile([128, 512], F32, tag="z")
                nc.tensor.matmul(
                    z_ps[:], w1a_m[:], xt[:, off:off + 512],
                    start=True, stop=False,
                )
                nc.tensor.matmul(
                    z_ps[:], w1b_m[:], xt[:, off + 512:off + 1024],
                    start=False, stop=True,
                )
                e_sb = ep.tile([128, 512], F32, tag="e")
                h_sb = hp.tile([128, 512], MM, tag="h")
                nc.scalar.activation(e_sb[:], z_ps[:], Exp, bias=b1s[:, :1])
                nc.scalar.activation(h_sb[:], e_sb[:], Ln, bias=1.0)
                mag_ps = magp.tile([2, 512], F32, tag="mag")
                nc.tensor.matmul(mag_ps[:], w2d_m[:], h_sb[:], start=True, stop=True)
                gi = g % MAG_GRP
                if gi == 0:
                    mag_sb = mp.tile([2, MAG_GRP * 512], F32, tag="magsb")
                nc.vector.tensor_copy(
                    mag_sb[:, gi * 512:(gi + 1) * 512], mag_ps[:]
                )
                if gi == MAG_GRP - 1:
                    nc.gpsimd.dma_start(mag_d[g // MAG_GRP], mag_sb[:])
    nc.compile()
    return nc


def _get_nc():
    if "nc" not in _CACHE:
        _CACHE["nc"] = _build_nc()
    return _CACHE["nc"]


def kernel(features, edge_vectors, edge_lengths, edge_index, w1, b1, w2, b2):
    global LAST_RESULTS
    from concourse.bass_utils import run_bass_kernel_spmd

    features = np.asarray(features, dtype=np.float32)
    edge_vectors = np.asarray(edge_vectors, dtype=np.float32)
    edge_lengths = np.asarray(edge_lengths, dtype=np.float32)
    edge_index = np.asarray(edge_index)
    w1 = np.asarray(w1, dtype=np.float32)
    b1 = np.asarray(b1, dtype=np.float32).reshape(-1)
    w2 = np.asarray(w2, dtype=np.float32).reshape(-1, 1)
    b2 = np.asarray(b2, dtype=np.float32).reshape(-1)

    # replicated small weights, padded for the stacked-z / block-diag tricks
    w1a = np.zeros((128, 128), np.float32)
    w1a[:, :H_DIM] = w1
    w1b = np.zeros((128, 128), np.float32)
    w1b[:, H_DIM:] = w1
    b1s = np.concatenate([b1, b1]).astype(np.float32).reshape(128, 1)
    w2d = np.zeros((128, 2), np.float32)
    w2d[:H_DIM, 0] = w2[:, 0]
    w2d[H_DIM:, 1] = w2[:, 0]

    # shard edges contiguously across cores; per-core transposed feature panel
    in_maps = []
    for c in range(N_CORES):
        sl = slice(c * EC, (c + 1) * EC)
        xt = np.zeros((128, ECP), np.float32)
        xt[:, :EC] = features[sl].T
        in_maps.append({"xt": xt, "w1a": w1a, "w1b": w1b, "b1s": b1s, "w2d": w2d})

    nc = _get_nc()
    res = run_bass_kernel_spmd(nc, in_maps, core_ids=list(range(N_CORES)))
    LAST_RESULTS = res

    # decode mag: out [NGRP, 2, MAG_GRP*512]; within group: col = s*512 + c,
    # value = edge g*4096 + row*512 + s*1024 + c
    mag = np.empty(E_TOT, np.float32)
    for c in range(N_CORES):
        arr = res.results[c]["mag"]  # [NGRP, 2, 2048]
        mc = arr.reshape(NGRP, 2, MAG_GRP, 512).transpose(0, 2, 1, 3).reshape(-1)
        mag[c * EC:(c + 1) * EC] = mc[:EC]

    # fold b2 and the shifted-softplus constant: h_ref = h_dev - log(2)
    mag = mag + (b2[0] - np.float32(np.log(2.0)) * w2.sum())

    center = edge_index[0].astype(np.int64)
    neigh = edge_index[1].astype(np.int64)

    # scatter-mean debias per center atom
    cnt = np.bincount(center, minlength=N_ATOMS).astype(np.float32)
    ssum = np.bincount(center, weights=mag.astype(np.float64), minlength=N_ATOMS)
    bias = (ssum / np.maximum(cnt, 1.0)).astype(np.float32)
    mag = mag - bias[center]

    # pair-averaged antisymmetric force assembly (see module docstring)
    unit = edge_vectors / edge_lengths[:, None]
    val = (0.5 * mag)[:, None] * unit  # [E, 3]
    forces = np.zeros((N_ATOMS, 3), np.float32)
    for k in range(3):
        fc = np.bincount(center, weights=val[:, k].astype(np.float64), minlength=N_ATOMS)
        fn = np.bincount(neigh, weights=val[:, k].astype(np.float64), minlength=N_ATOMS)
        forces[:, k] = (fc - fn).astype(np.float32)
    return forces
